# revision 6
# baseline (speedup 1.0000x reference)
"""Trainium2 Bass kernel for nn_BigramLM_34273839022823.

10-layer dense transformer LM forward: B=64, T=256, E=384, H=6, HS=64, V=65.
Sharding: data-parallel over batch across 8 NeuronCores (8 sequences each),
parameters replicated. No collectives.

Per-core design:
  - 2048 tokens as 16 tiles of 128 (seq s = tiles 2s, 2s+1); residual stream
    x_sb [128, 16, 384] stays SBUF-resident in natural (token-partition) form.
  - Matmuls run on transposed activations (hT [E, tok]) produced by PE
    transposes of the LN output; weights stream from DRAM in natural layout.
  - Attention uses transposed scores sT[tk, tq] = kT.T @ qT so causal mask +
    exp run as one ACT op + one affine_select (scores are tiny, so softmax
    without max-subtraction is exact enough); softmax denominators come from a
    ones-column appended to V; normalization multiplies by a K=1-matmul
    broadcast of 1/s.
  - All matmul inputs are float32r (PE full rate at N>=256).
  - LN scale/bias are folded into adjacent weights host-side; all additive
    bias terms are zero for this model instance (asserted), except b1 which
    is supported via the ACT per-partition bias port on the ReLU.
"""
import sys, os, contextlib

for _p in ("/opt/trn_rl_repo",):
    if _p not in sys.path and os.path.isdir(_p):
        sys.path.insert(0, _p)

import numpy as np

import concourse.bass as bass
import concourse.mybir as mybir
import concourse.tile as tile
from concourse.bass_utils import run_bass_kernel_spmd

F32 = mybir.dt.float32
F32R = mybir.dt.float32r
AF = mybir.ActivationFunctionType
ALU = mybir.AluOpType

N_CORES = 8
B, T, E, H, HS, L, V = 64, 256, 384, 6, 64, 10, 65
E4 = 4 * E                      # 1536
SEQ_PER_CORE = B // N_CORES     # 8
NTOK = SEQ_PER_CORE * T         # 2048
NTILE = NTOK // 128             # 16
KC = E // 128                   # 3 K-chunks over E
MC = E4 // 128                  # 12 M-chunks over 4E
SCALE = float(E) ** -0.5
EPS = 1e-5


def _legalize_waits(nc, max_waits=1):
    """walrus codegen in this toolchain accepts at most one sync wait per
    engine instruction; spill extras onto preceding single-wait NoOps."""
    n_split = 0
    for fn in nc.m.functions:
        for bb in fn.blocks:
            if not any(i.sync_info is not None and len(i.sync_info.on_wait) > max_waits
                       for i in bb.instructions):
                continue
            out = []
            for inst in bb.instructions:
                si = inst.sync_info
                if si is not None and len(si.on_wait) > max_waits:
                    waits = list(si.on_wait)
                    spill, keep = waits[:-max_waits], waits[-max_waits:]
                    for i, w in enumerate(spill):
                        out.append(mybir.InstNoOp(
                            name=f"{inst.name}-wsplit{i}",
                            engine=inst.engine,
                            sync_info=mybir.SyncInfo(on_wait=[w], on_update=[]),
                        ))
                        n_split += 1
                    inst.sync_info = mybir.SyncInfo(
                        on_wait=keep, on_update=list(si.on_update))
                out.append(inst)
            bb.instructions = out
    return n_split


def build(has_b1):
    nc = bass.Bass("TRN2", num_devices=N_CORES)

    # ---- DRAM I/O (per core) ----
    onehot_d = nc.dram_tensor("onehot", [V, NTOK], F32R, kind="ExternalInput")
    tokemb_d = nc.dram_tensor("tokemb", [V, E], F32R, kind="ExternalInput")
    posemb_d = nc.dram_tensor("posemb", [T, E], F32R, kind="ExternalInput")
    wqkv_d = nc.dram_tensor("wqkv", [L, 3, E, E], F32R, kind="ExternalInput")
    wo_d = nc.dram_tensor("wo", [L, E, E], F32R, kind="ExternalInput")
    w1_d = nc.dram_tensor("w1", [L, E, E4], F32R, kind="ExternalInput")
    w2_d = nc.dram_tensor("w2", [L, E4, E], F32R, kind="ExternalInput")
    wout_d = nc.dram_tensor("wout", [E, V + 1], F32R, kind="ExternalInput")
    ident_d = nc.dram_tensor("ident", [128, 128], F32R, kind="ExternalInput")
    ones_d = nc.dram_tensor("ones", [1, 128], F32R, kind="ExternalInput")
    onescol_d = nc.dram_tensor("onescol", [128, 1], F32R, kind="ExternalInput")
    b1_d = (nc.dram_tensor("b1t", [L, E4], F32, kind="ExternalInput")
            if has_b1 else None)
    logits_d = nc.dram_tensor("logits", [NTOK, V], F32, kind="ExternalOutput")

    with tile.TileContext(nc) as tc:
        with contextlib.ExitStack() as ctx:
            singles = ctx.enter_context(tc.tile_pool(name="singles", bufs=1))
            wpool = ctx.enter_context(tc.tile_pool(name="wpool", bufs=1))
            hpool = ctx.enter_context(tc.tile_pool(name="hpool", bufs=4))
            h1pool = ctx.enter_context(tc.tile_pool(name="h1pool", bufs=2))
            apool = ctx.enter_context(tc.tile_pool(name="apool", bufs=3))
            spool = ctx.enter_context(tc.tile_pool(name="spool", bufs=2))
            ps = ctx.enter_context(tc.tile_pool(name="ps", bufs=3, space="PSUM"))
            psa = ctx.enter_context(tc.tile_pool(name="psa", bufs=3, space="PSUM"))
            pst = ctx.enter_context(tc.tile_pool(name="pst", bufs=2, space="PSUM"))

            # ---- constants ----
            ident = singles.tile([128, 128], F32R)
            nc.sync.dma_start(out=ident, in_=ident_d.ap())
            ones = singles.tile([1, 128], F32R)
            nc.sync.dma_start(out=ones, in_=ones_d.ap())
            onescol = singles.tile([128, 1], F32R)
            nc.sync.dma_start(out=onescol, in_=onescol_d.ap())
            tokemb = singles.tile([V, E], F32R)
            nc.sync.dma_start(out=tokemb, in_=tokemb_d.ap())
            posemb = singles.tile([128, 2, E], F32R)
            nc.sync.dma_start(out=posemb,
                              in_=posemb_d.ap().rearrange("(h p) e -> p h e", p=128))
            onehot = singles.tile([V, NTOK], F32R)
            nc.sync.dma_start(out=onehot, in_=onehot_d.ap())
            wout = singles.tile([128, KC, V + 1], F32R)
            nc.sync.dma_start(out=wout,
                              in_=wout_d.ap().rearrange("(ko ki) v -> ki ko v", ki=128))
            b1t = None
            if has_b1:
                b1t = singles.tile([128, L, MC], F32)
                nc.sync.dma_start(
                    out=b1t, in_=b1_d.ap().rearrange("l (m p) -> p l m", p=128))

            # persistent residual stream
            x_sb = singles.tile([128, NTILE, E], F32)
            eps_sb = singles.tile([128, 1], F32)
            nc.vector.memset(eps_sb, EPS)

            # ---- embedding: x = onehot.T @ tokemb + pos_emb ----
            for t in range(NTILE):
                p0 = ps.tile([128, 512], F32, tag="p512", name="p512")
                nc.tensor.matmul(p0[:, :E], onehot[:, t * 128:(t + 1) * 128],
                                 tokemb, start=True, stop=False)
                nc.tensor.matmul(p0[:, :E], ident, posemb[:, t % 2, :],
                                 start=False, stop=True)
                nc.scalar.copy(out=x_sb[:, t, :], in_=p0[:, :E])

            def layer_norm_to_hT(x_src):
                """LN over the free (E) dim of x_src tiles -> list of 8
                transposed f32r tiles [128, KC, 256], one per 256-tok group."""
                mv = spool.tile([128, NTILE, 2], F32, tag="ln_mv", name="ln_mv")
                for t in range(NTILE):
                    st6 = spool.tile([128, 6], F32, tag="ln_st", name="ln_st")
                    nc.vector.bn_stats(out=st6, in_=x_src[:, t, :])
                    nc.vector.bn_aggr(out=mv[:, t, :], in_=st6)
                sd = spool.tile([128, NTILE], F32, tag="ln_sd", name="ln_sd")
                nc.scalar.activation(out=sd, in_=mv[:, :, 1], func=AF.Sqrt,
                                     bias=eps_sb[:, 0:1], scale=1.0)
                rv = spool.tile([128, NTILE], F32, tag="ln_rv", name="ln_rv")
                nc.vector.reciprocal(out=rv, in_=sd)
                nmr = spool.tile([128, NTILE], F32, tag="ln_nmr", name="ln_nmr")
                nc.vector.scalar_tensor_tensor(
                    out=nmr, in0=mv[:, :, 0], scalar=-1.0, in1=rv,
                    op0=ALU.mult, op1=ALU.mult)
                hts = []
                for g in range(8):
                    ht = hpool.tile([128, KC, 256], F32R, tag="hT", name="hT")
                    for half in range(2):
                        t = 2 * g + half
                        htmp = spool.tile([128, E], F32R, tag="htmp", name="htmp")
                        nc.scalar.activation(out=htmp, in_=x_src[:, t, :],
                                             func=AF.Identity,
                                             bias=nmr[:, t:t + 1],
                                             scale=rv[:, t:t + 1])
                        for k in range(KC):
                            ptr = pst.tile([128, 256], F32R, tag="ptr", name="ptr")
                            nc.tensor.transpose(
                                ptr[:, :128], htmp[:, k * 128:(k + 1) * 128], ident)
                            nc.scalar.copy(
                                out=ht[:, k, half * 128:(half + 1) * 128],
                                in_=ptr[:, :128])
                    hts.append(ht)
                return hts

            for l in range(L):
                # ---- layer weights (natural layout, K-chunked on partitions) --
                def wtile(tag, src_ap, shape, split):
                    w = wpool.tile(shape, F32R, tag=tag, name=tag)
                    nc.sync.dma_start(out=w, in_=src_ap.rearrange(
                        f"({split} ki) f -> ki {split} f", ki=128))
                    return w
                wq = wtile("wq", wqkv_d.ap()[l, 0], [128, KC, E], "ko")
                wk = wtile("wk", wqkv_d.ap()[l, 1], [128, KC, E], "ko")
                wv = wtile("wv", wqkv_d.ap()[l, 2], [128, KC, E], "ko")
                wo = wtile("wo", wo_d.ap()[l], [128, KC, E], "ko")
                w1 = wtile("w1", w1_d.ap()[l], [128, KC, E4], "ko")
                w2 = wtile("w2", w2_d.ap()[l], [128, MC, E], "mo")

                # ---- LN1 -> hT ----
                hts = layer_norm_to_hT(x_sb)

                # ---- attention, per sequence ----
                for s in range(SEQ_PER_CORE):
                    ht = hts[s]
                    # v (natural, per token tile) with appended ones column
                    v_sb = apool.tile([128, 2, H, HS + 1], F32R,
                                      tag="v_sb", name="v_sb")
                    nc.vector.tensor_copy(
                        out=v_sb[:, :, :, HS:HS + 1],
                        in_=onescol[:, 0:1, None, None]
                        .to_broadcast([128, 2, H, 1]))
                    for j in range(2):
                        pv = ps.tile([128, 512], F32, tag="p512", name="p512")
                        for k in range(KC):
                            nc.tensor.matmul(
                                pv[:, :E], ht[:, k, j * 128:(j + 1) * 128],
                                wv[:, k, :], start=(k == 0), stop=(k == KC - 1))
                        nc.vector.tensor_copy(
                            out=v_sb[:, j, :, 0:HS],
                            in_=pv[:, :E].rearrange("p (h d) -> p h d", h=H))
                    # qT, kT per head-pair: [128 rows = 2 heads x 64 dims, 256 tok]
                    qk = []
                    for c in range(KC):
                        pqk = ps.tile([128, 512], F32, tag="p512", name="p512")
                        for k in range(KC):
                            nc.tensor.matmul(
                                pqk[:, 0:256], wq[:, k, c * 128:(c + 1) * 128],
                                ht[:, k, :], start=(k == 0), stop=(k == KC - 1))
                        for k in range(KC):
                            nc.tensor.matmul(
                                pqk[:, 256:512], wk[:, k, c * 128:(c + 1) * 128],
                                ht[:, k, :], start=(k == 0), stop=(k == KC - 1))
                        qkt = apool.tile([128, 2, 256], F32R, tag="qkt", name="qkt")
                        nc.vector.tensor_copy(out=qkt, in_=pqk)
                        qk.append(qkt)

                    attn_cat = apool.tile([128, KC, 256], F32R,
                                          tag="attn_cat", name="attn_cat")
                    for c in range(KC):          # head pair
                        qkt = qk[c]
                        recf = apool.tile([1, 2, 256], F32, tag="recf", name="recf")
                        pats = []
                        for hh in range(2):
                            r0 = hh * 64
                            psc = ps.tile([128, 512], F32, tag="p512", name="p512")
                            for j in range(2):
                                nc.tensor.matmul(
                                    psc[:, j * 256:(j + 1) * 256],
                                    qkt[r0:r0 + 64, 1, j * 128:(j + 1) * 128],
                                    qkt[r0:r0 + 64, 0, :],
                                    start=True, stop=True)
                            expt = apool.tile([128, 2, 256], F32R,
                                              tag="expt", name="expt")
                            nc.scalar.activation(
                                out=expt.rearrange("p a b -> p (a b)"),
                                in_=psc, func=AF.Exp, scale=SCALE)
                            # keep where tq >= tk  (tk = 128*j + partition)
                            nc.gpsimd.affine_select(
                                out=expt, in_=expt, compare_op=ALU.is_ge,
                                fill=0.0, base=0, channel_multiplier=-1,
                                pattern=[[-128, 2], [1, 256]])
                            pat = psa.tile([HS + 1, 256], F32, tag="pat", name="pat")
                            for j in range(2):
                                nc.tensor.matmul(
                                    pat, v_sb[:, j, 2 * c + hh, :], expt[:, j, :],
                                    start=(j == 0), stop=(j == 1))
                            nc.vector.reciprocal(out=recf[:, hh, :],
                                                 in_=pat[HS:HS + 1, :])
                            pats.append(pat)
                        recr = apool.tile([1, 2, 256], F32R, tag="recr", name="recr")
                        nc.vector.tensor_copy(out=recr, in_=recf)
                        pbc = ps.tile([128, 512], F32, tag="p512", name="p512")
                        nc.tensor.matmul(
                            pbc[0:64, :], ones[:, 0:64],
                            recr.rearrange("o h t -> o (h t)"),
                            start=True, stop=True)
                        bcs = apool.tile([64, 512], F32, tag="bcs", name="bcs")
                        nc.scalar.copy(out=bcs, in_=pbc[0:64, :])
                        for hh in range(2):
                            nc.vector.tensor_mul(
                                out=attn_cat[hh * 64:(hh + 1) * 64, c, :],
                                in0=pats[hh][0:HS, :],
                                in1=bcs[:, hh * 256:(hh + 1) * 256])
                    # Wo + residual
                    for half in range(2):
                        t = 2 * s + half
                        po = ps.tile([128, 512], F32, tag="p512", name="p512")
                        for k in range(KC):
                            nc.tensor.matmul(
                                po[:, :E],
                                attn_cat[:, k, half * 128:(half + 1) * 128],
                                wo[:, k, :], start=(k == 0), stop=(k == KC - 1))
                        nc.vector.tensor_add(out=x_sb[:, t, :],
                                             in0=x_sb[:, t, :], in1=po[:, :E])

                # ---- MLP ----
                h2ts = layer_norm_to_hT(x_sb)
                for g in range(8):
                    h2t = h2ts[g]
                    h1t = h1pool.tile([128, MC, 256], F32R, tag="h1t", name="h1t")
                    for m in range(MC):
                        pm = ps.tile([128, 512], F32, tag="p512", name="p512")
                        for k in range(KC):
                            nc.tensor.matmul(
                                pm[:, 0:256], w1[:, k, m * 128:(m + 1) * 128],
                                h2t[:, k, :], start=(k == 0), stop=(k == KC - 1))
                        if has_b1:
                            nc.scalar.activation(out=h1t[:, m, :], in_=pm[:, 0:256],
                                                 func=AF.Relu,
                                                 bias=b1t[:, l, m:m + 1], scale=1.0)
                        else:
                            nc.scalar.activation(out=h1t[:, m, :], in_=pm[:, 0:256],
                                                 func=AF.Relu)
                    for half in range(2):
                        t = 2 * g + half
                        p2 = ps.tile([128, 512], F32, tag="p512", name="p512")
                        for m in range(MC):
                            nc.tensor.matmul(
                                p2[:, :E], h1t[:, m, half * 128:(half + 1) * 128],
                                w2[:, m, :], start=(m == 0), stop=(m == MC - 1))
                        nc.vector.tensor_add(out=x_sb[:, t, :],
                                             in0=x_sb[:, t, :], in1=p2[:, :E])

            # ---- final LN + unembed ----
            hfts = layer_norm_to_hT(x_sb)
            for t in range(NTILE):
                hft = hfts[t // 2]
                half = t % 2
                pl = ps.tile([128, 512], F32, tag="p512", name="p512")
                for k in range(KC):
                    nc.tensor.matmul(
                        pl[:, :V + 1], hft[:, k, half * 128:(half + 1) * 128],
                        wout[:, k, :], start=(k == 0), stop=(k == KC - 1))
                lsb = spool.tile([128, V], F32, tag="lsb", name="lsb")
                nc.scalar.copy(out=lsb, in_=pl[:, :V])
                nc.sync.dma_start(out=logits_d.ap()[t * 128:(t + 1) * 128, :],
                                  in_=lsb)

    _legalize_waits(nc)
    return nc


_CACHE = {}


def _get_nc(has_b1):
    if has_b1 not in _CACHE:
        _CACHE[has_b1] = build(has_b1)
    return _CACHE[has_b1]


def kernel(encoding, tok_emb, pos_emb, Wq, Wk, Wv, Wo, bo, W1, b1, W2, b2,
           ln1_s, ln1_b, ln2_s, ln2_b, lnf_s, lnf_b, Wout, bout):
    encoding = np.asarray(encoding)
    f = lambda a: np.ascontiguousarray(np.asarray(a), dtype=np.float32)
    tok_emb, pos_emb = f(tok_emb), f(pos_emb)
    Wq, Wk, Wv, Wo, bo = f(Wq), f(Wk), f(Wv), f(Wo), f(bo)
    W1, b1, W2, b2 = f(W1), f(b1), f(W2), f(b2)
    ln1_s, ln1_b, ln2_s, ln2_b = f(ln1_s), f(ln1_b), f(ln2_s), f(ln2_b)
    lnf_s, lnf_b, Wout, bout = f(lnf_s), f(lnf_b), f(Wout), f(bout)

    # --- host-side folding of LN affine params into adjacent matmuls ---
    # h = x_hat*g + b ; h @ W = x_hat @ (g[:,None]*W) + b @ W.
    def fold_qkv(W, g):
        Wt = W.transpose(0, 2, 1, 3).reshape(L, E, E)     # [L, E, (h d)]
        return Wt * g[:, :, None]

    wq_f = fold_qkv(Wq, ln1_s)
    wk_f = fold_qkv(Wk, ln1_s)
    wv_f = fold_qkv(Wv, ln1_s)
    w1_f = W1 * ln2_s[:, :, None]
    wout_f = Wout * lnf_s[:, None]

    def rank1(beta, W):  # [L,E] x [L,E,F] -> [L,F]
        return np.einsum('le,lef->lf', beta, W)

    q_bias = rank1(ln1_b, wq_f)
    k_bias = rank1(ln1_b, wk_f)
    v_bias = rank1(ln1_b, wv_f)
    w1_bias = rank1(ln2_b, w1_f) + b1
    out_bias = (lnf_b @ wout_f) + bout
    assert not q_bias.any() and not k_bias.any() and not v_bias.any(), \
        "nonzero folded q/k/v bias unsupported by this kernel build"
    assert not bo.any() and not b2.any(), "nonzero bo/b2 unsupported"
    assert not out_bias.any(), "nonzero unembed bias unsupported"
    has_b1 = bool(w1_bias.any())

    wqkv = np.ascontiguousarray(np.stack([wq_f, wk_f, wv_f], axis=1))
    wout_pad = np.zeros((E, V + 1), np.float32)
    wout_pad[:, :V] = wout_f
    nc = _get_nc(has_b1)

    ident_np = np.eye(128, dtype=np.float32)
    ones_np = np.ones((1, 128), np.float32)
    onescol_np = np.ones((128, 1), np.float32)
    enc_i = encoding.astype(np.int64)

    in_maps = []
    for c in range(N_CORES):
        enc_c = enc_i[c * SEQ_PER_CORE:(c + 1) * SEQ_PER_CORE].reshape(-1)
        onehot = (np.arange(V)[:, None] == enc_c[None, :]).astype(np.float32)
        m = {
            "onehot": onehot,
            "tokemb": tok_emb,
            "posemb": pos_emb,
            "wqkv": wqkv,
            "wo": Wo,
            "w1": w1_f,
            "w2": W2,
            "wout": wout_pad,
            "ident": ident_np,
            "ones": ones_np,
            "onescol": onescol_np,
        }
        if has_b1:
            m["b1t"] = np.ascontiguousarray(w1_bias)
        in_maps.append(m)

    res = run_bass_kernel_spmd(nc, in_maps, core_ids=list(range(N_CORES)))
    out = np.concatenate(
        [r["logits"].reshape(SEQ_PER_CORE, T, V) for r in res.results], axis=0)
    return out.astype(np.float32)


# revision 18
# speedup vs baseline: 1.0807x; 1.0807x over previous
"""Trainium2 Bass kernel for nn_BigramLM_34273839022823.

10-layer dense transformer LM forward: B=64, T=256, E=384, H=6, HS=64, V=65.
Sharding: data-parallel over batch across 8 NeuronCores (8 sequences each),
parameters replicated. No collectives.

Per-core design:
  - 2048 tokens as 16 tiles of 128 (seq s = tiles 2s, 2s+1); residual stream
    x_sb [128, 16, 384] stays SBUF-resident in natural (token-partition) form.
  - Matmuls run on transposed activations (hT [E, tok]) produced by PE
    transposes of the LN output; weights stream from DRAM in natural layout.
  - Attention uses transposed scores sT[tk, tq] = kT.T @ qT so causal mask +
    exp run as one ACT op + one affine_select (scores are tiny, so softmax
    without max-subtraction is exact enough); softmax denominators come from a
    ones-column appended to V; normalization multiplies by a K=1-matmul
    broadcast of 1/s.
  - All matmul inputs are float32r (PE full rate at N>=256).
  - LN scale/bias are folded into adjacent weights host-side; all additive
    bias terms are zero for this model instance (asserted), except b1 which
    is supported via the ACT per-partition bias port on the ReLU.
"""
import sys, os, contextlib

for _p in ("/opt/trn_rl_repo",):
    if _p not in sys.path and os.path.isdir(_p):
        sys.path.insert(0, _p)

import numpy as np

import concourse.bass as bass
import concourse.mybir as mybir
import concourse.tile as tile
from concourse.bass_utils import run_bass_kernel_spmd

F32 = mybir.dt.float32
F32R = mybir.dt.float32r
AF = mybir.ActivationFunctionType
ALU = mybir.AluOpType

N_CORES = 8
B, T, E, H, HS, L, V = 64, 256, 384, 6, 64, 10, 65
E4 = 4 * E                      # 1536
SEQ_PER_CORE = B // N_CORES     # 8
NTOK = SEQ_PER_CORE * T         # 2048
NTILE = NTOK // 128             # 16
KC = E // 128                   # 3 K-chunks over E
MC = E4 // 128                  # 12 M-chunks over 4E
SCALE = float(E) ** -0.5
EPS = 1e-5


def _legalize_waits(nc, max_waits=1):
    """walrus codegen in this toolchain accepts at most one sync wait per
    engine instruction; spill extras onto preceding single-wait NoOps."""
    n_split = 0
    for fn in nc.m.functions:
        for bb in fn.blocks:
            if not any(i.sync_info is not None and len(i.sync_info.on_wait) > max_waits
                       for i in bb.instructions):
                continue
            out = []
            for inst in bb.instructions:
                si = inst.sync_info
                if si is not None and len(si.on_wait) > max_waits:
                    waits = list(si.on_wait)
                    spill, keep = waits[:-max_waits], waits[-max_waits:]
                    for i, w in enumerate(spill):
                        out.append(mybir.InstNoOp(
                            name=f"{inst.name}-wsplit{i}",
                            engine=inst.engine,
                            sync_info=mybir.SyncInfo(on_wait=[w], on_update=[]),
                        ))
                        n_split += 1
                    inst.sync_info = mybir.SyncInfo(
                        on_wait=keep, on_update=list(si.on_update))
                out.append(inst)
            bb.instructions = out
    return n_split


def build(has_b1):
    nc = bass.Bass("TRN2", num_devices=N_CORES)

    # ---- DRAM I/O (per core) ----
    onehot_d = nc.dram_tensor("onehot", [V, NTOK], F32R, kind="ExternalInput")
    tokemb_d = nc.dram_tensor("tokemb", [V, E], F32R, kind="ExternalInput")
    posemb_d = nc.dram_tensor("posemb", [T, E], F32R, kind="ExternalInput")
    wqkv_d = nc.dram_tensor("wqkv", [L, 3, E, E], F32R, kind="ExternalInput")
    wo_d = nc.dram_tensor("wo", [L, E, E], F32R, kind="ExternalInput")
    w1_d = nc.dram_tensor("w1", [L, E, E4], F32R, kind="ExternalInput")
    w2_d = nc.dram_tensor("w2", [L, E4, E], F32R, kind="ExternalInput")
    wout_d = nc.dram_tensor("wout", [E, V + 1], F32R, kind="ExternalInput")
    ident_d = nc.dram_tensor("ident", [128, 128], F32R, kind="ExternalInput")
    onescol_d = nc.dram_tensor("onescol", [128, 1], F32R, kind="ExternalInput")
    ones_d = nc.dram_tensor("ones", [1, 128], F32R, kind="ExternalInput")
    b1_d = (nc.dram_tensor("b1t", [L, E4], F32, kind="ExternalInput")
            if has_b1 else None)
    logits_d = nc.dram_tensor("logits", [NTOK, V], F32, kind="ExternalOutput")

    with tile.TileContext(nc) as tc:
        with contextlib.ExitStack() as ctx:
            singles = ctx.enter_context(tc.tile_pool(name="singles", bufs=1))
            wpool = ctx.enter_context(tc.tile_pool(name="wpool", bufs=1))
            hpool = ctx.enter_context(tc.tile_pool(name="hpool", bufs=4))
            h1pool = ctx.enter_context(tc.tile_pool(name="h1pool", bufs=2))
            apool = ctx.enter_context(tc.tile_pool(name="apool", bufs=3))
            spool = ctx.enter_context(tc.tile_pool(name="spool", bufs=2))
            ps = ctx.enter_context(tc.tile_pool(name="ps", bufs=2, space="PSUM"))
            psat = ctx.enter_context(tc.tile_pool(name="psat", bufs=4, space="PSUM"))
            pst = ctx.enter_context(tc.tile_pool(name="pst", bufs=2, space="PSUM"))

            # ---- constants ----
            ident = singles.tile([128, 128], F32R)
            nc.sync.dma_start(out=ident, in_=ident_d.ap())
            onescol = singles.tile([128, 1], F32R)
            nc.sync.dma_start(out=onescol, in_=onescol_d.ap())
            ones = singles.tile([1, 128], F32R)
            nc.sync.dma_start(out=ones, in_=ones_d.ap())
            tokemb = singles.tile([V, E], F32R)
            nc.sync.dma_start(out=tokemb, in_=tokemb_d.ap())
            posemb = singles.tile([128, 2, E], F32R)
            nc.sync.dma_start(out=posemb,
                              in_=posemb_d.ap().rearrange("(h p) e -> p h e", p=128))
            onehot = singles.tile([V, NTOK], F32R)
            nc.sync.dma_start(out=onehot, in_=onehot_d.ap())
            wout = singles.tile([128, KC, V + 1], F32R)
            nc.sync.dma_start(out=wout,
                              in_=wout_d.ap().rearrange("(ko ki) v -> ki ko v", ki=128))
            b1t = None
            if has_b1:
                b1t = singles.tile([128, L, MC], F32)
                nc.sync.dma_start(
                    out=b1t, in_=b1_d.ap().rearrange("l (m p) -> p l m", p=128))

            # persistent residual stream
            x_sb = singles.tile([128, NTILE, E], F32)
            eps_sb = singles.tile([128, 1], F32)
            nc.vector.memset(eps_sb, EPS)

            # ---- embedding: x = onehot.T @ tokemb + pos_emb ----
            for t in range(NTILE):
                p0 = ps.tile([128, 512], F32, tag="p512", name="p512")
                nc.tensor.matmul(p0[:, :E], onehot[:, t * 128:(t + 1) * 128],
                                 tokemb, start=True, stop=False)
                nc.tensor.matmul(p0[:, :E], ident, posemb[:, t % 2, :],
                                 start=False, stop=True)
                nc.scalar.copy(out=x_sb[:, t, :], in_=p0[:, :E])

            def layer_norm_to_hT(x_src):
                """LN over the free (E) dim of x_src tiles -> list of 8
                transposed f32r tiles [128, KC, 256], one per 256-tok group."""
                mv = spool.tile([128, NTILE, 2], F32, tag="ln_mv", name="ln_mv")
                for t in range(NTILE):
                    st6 = spool.tile([128, 6], F32, tag="ln_st", name="ln_st")
                    nc.vector.bn_stats(out=st6, in_=x_src[:, t, :])
                    nc.vector.bn_aggr(out=mv[:, t, :], in_=st6)
                # rsqrt(var+eps) = exp(-0.5*ln(var+eps)); Ln/Exp share one
                # ACT table set (unlike Sqrt), avoiding 1.3us table reloads.
                lnv = spool.tile([128, NTILE], F32, tag="ln_lnv", name="ln_lnv")
                nc.scalar.activation(out=lnv, in_=mv[:, :, 1], func=AF.Ln,
                                     bias=eps_sb[:, 0:1], scale=1.0)
                rv = spool.tile([128, NTILE], F32, tag="ln_rv", name="ln_rv")
                nc.scalar.activation(out=rv, in_=lnv, func=AF.Exp, scale=-0.5)
                hts = []
                htmps = []
                for t in range(NTILE):
                    htmp = spool.tile([128, E], F32R, tag="htmp", name="htmp",
                                      bufs=4)
                    nc.vector.tensor_scalar(
                        out=htmp, in0=x_src[:, t, :],
                        scalar1=mv[:, t:t + 1, 0], scalar2=rv[:, t:t + 1],
                        op0=ALU.subtract, op1=ALU.mult)
                    htmps.append(htmp)
                for g in range(8):
                    ht = hpool.tile([128, KC, 256], F32R, tag="hT", name="hT")
                    ptrA = pst.tile([128, 512], F32R, tag="ptr", name="ptr")
                    ptrB = pst.tile([128, 512], F32R, tag="ptr", name="ptr")
                    for half in range(2):
                        htmp = htmps[2 * g + half]
                        for k in range(KC):
                            dst = (ptrA[:, (2 * k + half) * 128:(2 * k + half + 1) * 128]
                                   if k < 2 else
                                   ptrB[:, half * 128:(half + 1) * 128])
                            nc.tensor.transpose(
                                dst, htmp[:, k * 128:(k + 1) * 128], ident)
                    nc.scalar.copy(out=ht[:, 0:2, :], in_=ptrA)
                    nc.scalar.copy(out=ht[:, 2, :], in_=ptrB[:, 0:256])
                    hts.append(ht)
                return hts

            for l in range(L):
                # ---- layer weights (natural layout, K-chunked on partitions) --
                def wtile(tag, src_ap, shape, split):
                    w = wpool.tile(shape, F32R, tag=tag, name=tag)
                    nc.sync.dma_start(out=w, in_=src_ap.rearrange(
                        f"({split} ki) f -> ki {split} f", ki=128))
                    return w
                wq = wtile("wq", wqkv_d.ap()[l, 0], [128, KC, E], "ko")
                wk = wtile("wk", wqkv_d.ap()[l, 1], [128, KC, E], "ko")
                wv = wtile("wv", wqkv_d.ap()[l, 2], [128, KC, E], "ko")
                wo = wtile("wo", wo_d.ap()[l], [128, KC, E], "ko")
                w1 = wtile("w1", w1_d.ap()[l], [128, KC, E4], "ko")
                w2 = wtile("w2", w2_d.ap()[l], [128, MC, E], "mo")

                # ---- LN1 -> hT ----
                hts = layer_norm_to_hT(x_sb)

                # ---- attention, per sequence ----
                for s in range(SEQ_PER_CORE):
                    ht = hts[s]
                    # v (natural, per token tile) with appended ones column
                    v_sb = apool.tile([128, 2, H, HS + 1], F32R,
                                      tag="v_sb", name="v_sb")
                    nc.vector.tensor_copy(
                        out=v_sb[:, :, :, HS:HS + 1],
                        in_=onescol[:, 0:1, None, None]
                        .to_broadcast([128, 2, H, 1]))
                    for j in range(2):
                        pv = ps.tile([128, 512], F32, tag="p512", name="p512")
                        for k in range(KC):
                            nc.tensor.matmul(
                                pv[:, :E], ht[:, k, j * 128:(j + 1) * 128],
                                wv[:, k, :], start=(k == 0), stop=(k == KC - 1))
                        nc.vector.tensor_copy(
                            out=v_sb[:, j, :, 0:HS],
                            in_=pv[:, :E].rearrange("p (h d) -> p h d", h=H))
                    # qT, kT per head-pair: [128 rows = 2 heads x 64 dims, 256 tok]
                    qk = []
                    for c in range(KC):
                        pqk = ps.tile([128, 512], F32, tag="p512", name="p512")
                        for k in range(KC):
                            nc.tensor.matmul(
                                pqk[:, 0:256], wq[:, k, c * 128:(c + 1) * 128],
                                ht[:, k, :], start=(k == 0), stop=(k == KC - 1))
                        for k in range(KC):
                            nc.tensor.matmul(
                                pqk[:, 256:512], wk[:, k, c * 128:(c + 1) * 128],
                                ht[:, k, :], start=(k == 0), stop=(k == KC - 1))
                        qkt = apool.tile([128, 2, 256], F32R, tag="qkt", name="qkt")
                        nc.scalar.copy(out=qkt, in_=pqk)
                        qk.append(qkt)

                    attn_cat = apool.tile([128, KC, 256], F32R,
                                          tag="attn_cat", name="attn_cat")
                    # phase 1: all 6 heads' transposed scores
                    expts = []
                    for h in range(H):
                        c, hh = h // 2, h % 2
                        qkt = qk[c]
                        r0 = hh * 64
                        psc = psat.tile([128, 512], F32, tag="psat", name="psat")
                        for j in range(2):
                            nc.tensor.matmul(
                                psc[:, j * 256:(j + 1) * 256],
                                qkt[r0:r0 + 64, 1, j * 128:(j + 1) * 128],
                                qkt[r0:r0 + 64, 0, :],
                                start=True, stop=True)
                        expt = apool.tile([128, 2, 256], F32R,
                                          tag="expt", name="expt", bufs=7)
                        nc.scalar.activation(
                            out=expt.rearrange("p a b -> p (a b)"),
                            in_=psc, func=AF.Exp, scale=SCALE)
                        # keep where tq >= tk  (tk = 128*j + partition)
                        nc.gpsimd.affine_select(
                            out=expt, in_=expt, compare_op=ALU.is_ge,
                            fill=0.0, base=0, channel_multiplier=-1,
                            pattern=[[-128, 2], [1, 256]])
                        expts.append(expt)
                    # phase 2: attnT + row-sums per head pair
                    recfs, pats = [], []
                    for c in range(KC):
                        recf = apool.tile([1, 2, 256], F32, tag="recf", name="recf")
                        pat = psat.tile([HS + 1, 512], F32, tag="psat", name="psat")
                        for hh in range(2):
                            expt = expts[2 * c + hh]
                            for j in range(2):
                                nc.tensor.matmul(
                                    pat[:, hh * 256:(hh + 1) * 256],
                                    v_sb[:, j, 2 * c + hh, :], expt[:, j, :],
                                    start=(j == 0), stop=(j == 1))
                        nc.vector.reciprocal(
                            out=recf.rearrange("p a b -> p (a b)"),
                            in_=pat[HS:HS + 1, :])
                        recfs.append(recf)
                        pats.append(pat)
                    # phase 3: broadcast 1/s (K=1 matmul) and normalize
                    recrs = []
                    for c in range(KC):
                        recr = apool.tile([1, 2, 256], F32R, tag="recr", name="recr")
                        nc.gpsimd.tensor_copy(out=recr, in_=recfs[c])
                        recrs.append(recr)
                    bcss = []
                    for c in range(KC):
                        pbc = psat.tile([64, 512], F32, tag="psat", name="pbc")
                        nc.tensor.matmul(
                            pbc, ones[:, 0:64],
                            recrs[c].rearrange("o h t -> o (h t)"),
                            start=True, stop=True)
                        bcs = apool.tile([64, 2, 256], F32, tag="bcs", name="bcs")
                        nc.scalar.copy(out=bcs.rearrange("p a b -> p (a b)"),
                                       in_=pbc)
                        bcss.append(bcs)
                    for c in range(KC):
                        for hh in range(2):
                            nc.vector.tensor_mul(
                                out=attn_cat[hh * 64:(hh + 1) * 64, c, :],
                                in0=pats[c][0:HS, hh * 256:(hh + 1) * 256],
                                in1=bcss[c][:, hh, :])
                    # Wo + residual
                    for half in range(2):
                        t = 2 * s + half
                        po = ps.tile([128, 512], F32, tag="p512", name="p512")
                        for k in range(KC):
                            nc.tensor.matmul(
                                po[:, :E],
                                attn_cat[:, k, half * 128:(half + 1) * 128],
                                wo[:, k, :], start=(k == 0), stop=(k == KC - 1))
                        nc.vector.tensor_add(out=x_sb[:, t, :],
                                             in0=x_sb[:, t, :], in1=po[:, :E])

                # ---- MLP ----
                h2ts = layer_norm_to_hT(x_sb)
                for g in range(8):
                    h2t = h2ts[g]
                    h1t = h1pool.tile([128, MC, 256], F32R, tag="h1t", name="h1t")
                    for m2 in range(MC // 2):
                        pm = ps.tile([128, 512], F32, tag="p512", name="p512")
                        for mm in range(2):
                            m = 2 * m2 + mm
                            for k in range(KC):
                                nc.tensor.matmul(
                                    pm[:, mm * 256:(mm + 1) * 256],
                                    w1[:, k, m * 128:(m + 1) * 128],
                                    h2t[:, k, :], start=(k == 0), stop=(k == KC - 1))
                        if has_b1:
                            for mm in range(2):
                                m = 2 * m2 + mm
                                nc.scalar.activation(
                                    out=h1t[:, m, :], in_=pm[:, mm * 256:(mm + 1) * 256],
                                    func=AF.Relu, bias=b1t[:, l, m:m + 1], scale=1.0)
                        else:
                            nc.scalar.activation(
                                out=h1t[:, 2 * m2:2 * m2 + 2, :], in_=pm,
                                func=AF.Relu)
                    for half in range(2):
                        t = 2 * g + half
                        p2 = ps.tile([128, 512], F32, tag="p512", name="p512")
                        for m in range(MC):
                            nc.tensor.matmul(
                                p2[:, :E], h1t[:, m, half * 128:(half + 1) * 128],
                                w2[:, m, :], start=(m == 0), stop=(m == MC - 1))
                        nc.vector.tensor_add(out=x_sb[:, t, :],
                                             in0=x_sb[:, t, :], in1=p2[:, :E])

            # ---- final LN + unembed ----
            hfts = layer_norm_to_hT(x_sb)
            for t in range(NTILE):
                hft = hfts[t // 2]
                half = t % 2
                pl = ps.tile([128, 512], F32, tag="p512", name="p512")
                for k in range(KC):
                    nc.tensor.matmul(
                        pl[:, :V + 1], hft[:, k, half * 128:(half + 1) * 128],
                        wout[:, k, :], start=(k == 0), stop=(k == KC - 1))
                lsb = spool.tile([128, V], F32, tag="lsb", name="lsb")
                nc.scalar.copy(out=lsb, in_=pl[:, :V])
                nc.sync.dma_start(out=logits_d.ap()[t * 128:(t + 1) * 128, :],
                                  in_=lsb)

    _legalize_waits(nc)
    return nc


_CACHE = {}


def _get_nc(has_b1):
    if has_b1 not in _CACHE:
        _CACHE[has_b1] = build(has_b1)
    return _CACHE[has_b1]


def kernel(encoding, tok_emb, pos_emb, Wq, Wk, Wv, Wo, bo, W1, b1, W2, b2,
           ln1_s, ln1_b, ln2_s, ln2_b, lnf_s, lnf_b, Wout, bout):
    encoding = np.asarray(encoding)
    f = lambda a: np.ascontiguousarray(np.asarray(a), dtype=np.float32)
    tok_emb, pos_emb = f(tok_emb), f(pos_emb)
    Wq, Wk, Wv, Wo, bo = f(Wq), f(Wk), f(Wv), f(Wo), f(bo)
    W1, b1, W2, b2 = f(W1), f(b1), f(W2), f(b2)
    ln1_s, ln1_b, ln2_s, ln2_b = f(ln1_s), f(ln1_b), f(ln2_s), f(ln2_b)
    lnf_s, lnf_b, Wout, bout = f(lnf_s), f(lnf_b), f(Wout), f(bout)

    # --- host-side folding of LN affine params into adjacent matmuls ---
    # h = x_hat*g + b ; h @ W = x_hat @ (g[:,None]*W) + b @ W.
    def fold_qkv(W, g):
        Wt = W.transpose(0, 2, 1, 3).reshape(L, E, E)     # [L, E, (h d)]
        return Wt * g[:, :, None]

    wq_f = fold_qkv(Wq, ln1_s)
    wk_f = fold_qkv(Wk, ln1_s)
    wv_f = fold_qkv(Wv, ln1_s)
    w1_f = W1 * ln2_s[:, :, None]
    wout_f = Wout * lnf_s[:, None]

    def rank1(beta, W):  # [L,E] x [L,E,F] -> [L,F]
        return np.einsum('le,lef->lf', beta, W)

    q_bias = rank1(ln1_b, wq_f)
    k_bias = rank1(ln1_b, wk_f)
    v_bias = rank1(ln1_b, wv_f)
    w1_bias = rank1(ln2_b, w1_f) + b1
    out_bias = (lnf_b @ wout_f) + bout
    assert not q_bias.any() and not k_bias.any() and not v_bias.any(), \
        "nonzero folded q/k/v bias unsupported by this kernel build"
    assert not bo.any() and not b2.any(), "nonzero bo/b2 unsupported"
    assert not out_bias.any(), "nonzero unembed bias unsupported"
    has_b1 = bool(w1_bias.any())

    wqkv = np.ascontiguousarray(np.stack([wq_f, wk_f, wv_f], axis=1))
    wout_pad = np.zeros((E, V + 1), np.float32)
    wout_pad[:, :V] = wout_f
    nc = _get_nc(has_b1)

    ident_np = np.eye(128, dtype=np.float32)
    onescol_np = np.ones((128, 1), np.float32)
    enc_i = encoding.astype(np.int64)

    in_maps = []
    for c in range(N_CORES):
        enc_c = enc_i[c * SEQ_PER_CORE:(c + 1) * SEQ_PER_CORE].reshape(-1)
        onehot = (np.arange(V)[:, None] == enc_c[None, :]).astype(np.float32)
        m = {
            "onehot": onehot,
            "tokemb": tok_emb,
            "posemb": pos_emb,
            "wqkv": wqkv,
            "wo": Wo,
            "w1": w1_f,
            "w2": W2,
            "wout": wout_pad,
            "ident": ident_np,
            "onescol": onescol_np,
            "ones": np.ones((1, 128), np.float32),
        }
        if has_b1:
            m["b1t"] = np.ascontiguousarray(w1_bias)
        in_maps.append(m)

    res = run_bass_kernel_spmd(nc, in_maps, core_ids=list(range(N_CORES)))
    out = np.concatenate(
        [r["logits"].reshape(SEQ_PER_CORE, T, V) for r in res.results], axis=0)
    return out.astype(np.float32)


# revision 22
# speedup vs baseline: 3648.7883x; 3376.1676x over previous
"""Trainium2 Bass kernel for nn_BigramLM_34273839022823.

10-layer dense transformer LM forward: B=64, T=256, E=384, H=6, HS=64, V=65.
Sharding: data-parallel over batch across 8 NeuronCores (8 sequences each),
parameters replicated. No collectives.

Per-core design:
  - 2048 tokens as 16 tiles of 128 (seq s = tiles 2s, 2s+1); residual stream
    x_sb [128, 16, 384] stays SBUF-resident in natural (token-partition) form.
  - Matmuls run on transposed activations (hT [E, tok]) produced by PE
    transposes of the LN output; weights stream from DRAM in natural layout.
  - Attention uses transposed scores sT[tk, tq] = kT.T @ qT so causal mask +
    exp run as one ACT op + one affine_select (scores are tiny, so softmax
    without max-subtraction is exact enough); softmax denominators come from a
    ones-column appended to V; normalization multiplies by a K=1-matmul
    broadcast of 1/s.
  - All matmul inputs are float32r (PE full rate at N>=256).
  - LN scale/bias are folded into adjacent weights host-side; all additive
    bias terms are zero for this model instance (asserted), except b1 which
    is supported via the ACT per-partition bias port on the ReLU.
"""
import sys, os, contextlib

for _p in ("/opt/trn_rl_repo",):
    if _p not in sys.path and os.path.isdir(_p):
        sys.path.insert(0, _p)

import numpy as np

import concourse.bass as bass
import concourse.mybir as mybir
import concourse.tile as tile
from concourse.bass_utils import run_bass_kernel_spmd

F32 = mybir.dt.float32
F32R = mybir.dt.float32r
AF = mybir.ActivationFunctionType
ALU = mybir.AluOpType

N_CORES = 8
B, T, E, H, HS, L, V = 64, 256, 384, 6, 64, 10, 65
E4 = 4 * E                      # 1536
SEQ_PER_CORE = B // N_CORES     # 8
NTOK = SEQ_PER_CORE * T         # 2048
NTILE = NTOK // 128             # 16
KC = E // 128                   # 3 K-chunks over E
MC = E4 // 128                  # 12 M-chunks over 4E
SCALE = float(E) ** -0.5
EPS = 1e-5


def _legalize_waits(nc, max_waits=1):
    """walrus codegen in this toolchain accepts at most one sync wait per
    engine instruction; spill extras onto preceding single-wait NoOps."""
    n_split = 0
    for fn in nc.m.functions:
        for bb in fn.blocks:
            if not any(i.sync_info is not None and len(i.sync_info.on_wait) > max_waits
                       for i in bb.instructions):
                continue
            out = []
            for inst in bb.instructions:
                si = inst.sync_info
                if si is not None and len(si.on_wait) > max_waits:
                    waits = list(si.on_wait)
                    spill, keep = waits[:-max_waits], waits[-max_waits:]
                    for i, w in enumerate(spill):
                        out.append(mybir.InstNoOp(
                            name=f"{inst.name}-wsplit{i}",
                            engine=inst.engine,
                            sync_info=mybir.SyncInfo(on_wait=[w], on_update=[]),
                        ))
                        n_split += 1
                    inst.sync_info = mybir.SyncInfo(
                        on_wait=keep, on_update=list(si.on_update))
                out.append(inst)
            bb.instructions = out
    return n_split


def build(has_b1, reps=1):
    nc = bass.Bass("TRN2", num_devices=N_CORES)

    # ---- DRAM I/O (per core) ----
    onehot_d = nc.dram_tensor("onehot", [V, NTOK], F32R, kind="ExternalInput")
    tokemb_d = nc.dram_tensor("tokemb", [V, E], F32R, kind="ExternalInput")
    posemb_d = nc.dram_tensor("posemb", [T, E], F32R, kind="ExternalInput")
    wqkv_d = nc.dram_tensor("wqkv", [L, 3, E, E], F32R, kind="ExternalInput")
    wo_d = nc.dram_tensor("wo", [L, E, E], F32R, kind="ExternalInput")
    w1_d = nc.dram_tensor("w1", [L, E, E4], F32R, kind="ExternalInput")
    w2_d = nc.dram_tensor("w2", [L, E4, E], F32R, kind="ExternalInput")
    wout_d = nc.dram_tensor("wout", [E, V + 1], F32R, kind="ExternalInput")
    ident_d = nc.dram_tensor("ident", [128, 128], F32R, kind="ExternalInput")
    onescol_d = nc.dram_tensor("onescol", [128, 1], F32R, kind="ExternalInput")
    ones_d = nc.dram_tensor("ones", [1, 128], F32R, kind="ExternalInput")
    b1_d = (nc.dram_tensor("b1t", [L, E4], F32, kind="ExternalInput")
            if has_b1 else None)
    logits_d = nc.dram_tensor("logits", [NTOK, V], F32, kind="ExternalOutput")

    with tile.TileContext(nc) as tc:
        with contextlib.ExitStack() as ctx:
            singles = ctx.enter_context(tc.tile_pool(name="singles", bufs=1))
            wpool = ctx.enter_context(tc.tile_pool(name="wpool", bufs=1))
            hpool = ctx.enter_context(tc.tile_pool(name="hpool", bufs=3))
            h1pool = ctx.enter_context(tc.tile_pool(name="h1pool", bufs=2))
            apool = ctx.enter_context(tc.tile_pool(name="apool", bufs=3))
            spool = ctx.enter_context(tc.tile_pool(name="spool", bufs=2))
            ps = ctx.enter_context(tc.tile_pool(name="ps", bufs=2, space="PSUM"))
            psat = ctx.enter_context(tc.tile_pool(name="psat", bufs=6, space="PSUM"))

            # ---- constants ----
            ident = singles.tile([128, 128], F32R)
            nc.sync.dma_start(out=ident, in_=ident_d.ap())
            onescol = singles.tile([128, 1], F32R)
            nc.sync.dma_start(out=onescol, in_=onescol_d.ap())
            ones = singles.tile([1, 128], F32R)
            nc.sync.dma_start(out=ones, in_=ones_d.ap())
            tokemb = singles.tile([V, E], F32R)
            nc.sync.dma_start(out=tokemb, in_=tokemb_d.ap())
            posemb = singles.tile([128, 2, E], F32R)
            nc.sync.dma_start(out=posemb,
                              in_=posemb_d.ap().rearrange("(h p) e -> p h e", p=128))
            onehot = singles.tile([V, NTOK], F32R)
            nc.sync.dma_start(out=onehot, in_=onehot_d.ap())
            wout = singles.tile([128, KC, V + 1], F32R)
            nc.sync.dma_start(out=wout,
                              in_=wout_d.ap().rearrange("(ko ki) v -> ki ko v", ki=128))
            b1t = None
            if has_b1:
                b1t = singles.tile([128, L, MC], F32)
                nc.sync.dma_start(
                    out=b1t, in_=b1_d.ap().rearrange("l (m p) -> p l m", p=128))

            # persistent residual stream
            x_sb = singles.tile([128, NTILE, E], F32)
            eps_sb = singles.tile([128, 1], F32)
            nc.vector.memset(eps_sb, EPS)

            rep_cm = tc.For_i(0, reps, 1) if reps > 1 else contextlib.nullcontext()
            rep_cm.__enter__()
            # ---- embedding: x = onehot.T @ tokemb + pos_emb ----
            for t in range(NTILE):
                p0 = ps.tile([128, 512], F32, tag="p512", name="p512")
                nc.tensor.matmul(p0[:, :E], onehot[:, t * 128:(t + 1) * 128],
                                 tokemb, start=True, stop=False)
                nc.tensor.matmul(p0[:, :E], ident, posemb[:, t % 2, :],
                                 start=False, stop=True)
                nc.scalar.copy(out=x_sb[:, t, :], in_=p0[:, :E])

            def layer_norm_to_hT(x_src):
                """LN over the free (E) dim of x_src tiles -> list of 8
                transposed f32r tiles [128, KC, 256], one per 256-tok group."""
                mv = spool.tile([128, NTILE, 2], F32, tag="ln_mv", name="ln_mv")
                for t in range(NTILE):
                    st6 = spool.tile([128, 6], F32, tag="ln_st", name="ln_st")
                    nc.vector.bn_stats(out=st6, in_=x_src[:, t, :])
                    nc.vector.bn_aggr(out=mv[:, t, :], in_=st6)
                # rsqrt(var+eps) = exp(-0.5*ln(var+eps)); Ln/Exp share one
                # ACT table set (unlike Sqrt), avoiding 1.3us table reloads.
                lnv = spool.tile([128, NTILE], F32, tag="ln_lnv", name="ln_lnv")
                nc.scalar.activation(out=lnv, in_=mv[:, :, 1], func=AF.Ln,
                                     bias=eps_sb[:, 0:1], scale=1.0)
                rv = spool.tile([128, NTILE], F32, tag="ln_rv", name="ln_rv")
                nc.scalar.activation(out=rv, in_=lnv, func=AF.Exp, scale=-0.5)
                hts = []
                htmps = []
                for t in range(NTILE):
                    htmp = spool.tile([128, E], F32R, tag="htmp", name="htmp",
                                      bufs=5)
                    nc.vector.tensor_scalar(
                        out=htmp, in0=x_src[:, t, :],
                        scalar1=mv[:, t:t + 1, 0], scalar2=rv[:, t:t + 1],
                        op0=ALU.subtract, op1=ALU.mult)
                    htmps.append(htmp)
                for g in range(4):          # 512-token groups (2 seqs)
                    ht = hpool.tile([128, KC, 512], F32R, tag="hT", name="hT")
                    for k in range(KC):
                        ptr = psat.tile([128, 512], F32R, tag="psat", name="ptr")
                        for q in range(4):
                            nc.tensor.transpose(
                                ptr[:, q * 128:(q + 1) * 128],
                                htmps[4 * g + q][:, k * 128:(k + 1) * 128], ident)
                        nc.scalar.copy(out=ht[:, k, :], in_=ptr)
                    hts.append(ht)
                return hts

            for l in range(L):
                # ---- layer weights (natural layout, K-chunked on partitions) --
                def wtile(tag, src_ap, shape, split):
                    w = wpool.tile(shape, F32R, tag=tag, name=tag)
                    nc.sync.dma_start(out=w, in_=src_ap.rearrange(
                        f"({split} ki) f -> ki {split} f", ki=128))
                    return w
                wq = wtile("wq", wqkv_d.ap()[l, 0], [128, KC, E], "ko")
                wk = wtile("wk", wqkv_d.ap()[l, 1], [128, KC, E], "ko")
                wv = wtile("wv", wqkv_d.ap()[l, 2], [128, KC, E], "ko")
                wo = wtile("wo", wo_d.ap()[l], [128, KC, E], "ko")
                w1 = wtile("w1", w1_d.ap()[l], [128, KC, E4], "ko")
                w2 = wtile("w2", w2_d.ap()[l], [128, MC, E], "mo")

                # ---- LN1 -> hT ----
                hts = layer_norm_to_hT(x_sb)

                # ---- attention, per sequence (Wo pipelined one seq behind) --
                qk_pair = {}
                pending_wo = []

                def emit_wo(s, attn_cat):
                    for half in range(2):
                        t = 2 * s + half
                        po = ps.tile([128, 512], F32, tag="p512", name="p512")
                        for k in range(KC):
                            nc.tensor.matmul(
                                po[:, :E],
                                attn_cat[:, k, half * 128:(half + 1) * 128],
                                wo[:, k, :], start=(k == 0), stop=(k == KC - 1))
                        nc.vector.tensor_add(out=x_sb[:, t, :],
                                             in0=x_sb[:, t, :], in1=po[:, :E])

                for s in range(SEQ_PER_CORE):
                    ht = hts[s // 2]
                    sw = s % 2
                    # v (natural, per token tile) with appended ones column
                    v_sb = apool.tile([128, 2, H, HS + 1], F32R,
                                      tag="v_sb", name="v_sb")
                    nc.vector.tensor_copy(
                        out=v_sb[:, :, :, HS:HS + 1],
                        in_=onescol[:, 0:1, None, None]
                        .to_broadcast([128, 2, H, 1]))
                    for j in range(2):
                        pv = ps.tile([128, 512], F32, tag="p512", name="p512")
                        for k in range(KC):
                            nc.tensor.matmul(
                                pv[:, :E],
                                ht[:, k, (sw * 2 + j) * 128:(sw * 2 + j + 1) * 128],
                                wv[:, k, :], start=(k == 0), stop=(k == KC - 1))
                        nc.vector.tensor_copy(
                            out=v_sb[:, j, :, 0:HS],
                            in_=pv[:, :E].rearrange("p (h d) -> p h d", h=H))
                    # qT, kT per head-pair for BOTH seqs of the group, N=512
                    if sw == 0:
                        qk = []
                        for c in range(KC):
                            pq = ps.tile([128, 512], F32, tag="p512", name="p512")
                            for k in range(KC):
                                nc.tensor.matmul(
                                    pq, wq[:, k, c * 128:(c + 1) * 128],
                                    ht[:, k, :], start=(k == 0), stop=(k == KC - 1))
                            pk = ps.tile([128, 512], F32, tag="p512", name="p512")
                            for k in range(KC):
                                nc.tensor.matmul(
                                    pk, wk[:, k, c * 128:(c + 1) * 128],
                                    ht[:, k, :], start=(k == 0), stop=(k == KC - 1))
                            qkt = apool.tile([128, 2, 512], F32R, tag="qkt",
                                             name="qkt")
                            nc.scalar.copy(out=qkt[:, 0, :], in_=pq)
                            nc.scalar.copy(out=qkt[:, 1, :], in_=pk)
                            qk.append(qkt)
                        qk_pair[s // 2] = qk
                    qk = qk_pair[s // 2]

                    attn_cat = apool.tile([128, KC, 256], F32R,
                                          tag="attn_cat", name="attn_cat")
                    # phase 1: all 6 heads' transposed scores
                    expts = []
                    for h in range(H):
                        c, hh = h // 2, h % 2
                        qkt = qk[c]
                        r0 = hh * 64
                        psc = psat.tile([128, 512], F32, tag="psat", name="psat")
                        for j in range(2):
                            nc.tensor.matmul(
                                psc[:, j * 256:(j + 1) * 256],
                                qkt[r0:r0 + 64, 1,
                                    sw * 256 + j * 128:sw * 256 + (j + 1) * 128],
                                qkt[r0:r0 + 64, 0, sw * 256:(sw + 1) * 256],
                                start=True, stop=True)
                        expt = apool.tile([128, 2, 256], F32R,
                                          tag="expt", name="expt", bufs=5)
                        nc.scalar.activation(
                            out=expt.rearrange("p a b -> p (a b)"),
                            in_=psc, func=AF.Exp, scale=SCALE)
                        # keep where tq >= tk  (tk = 128*j + partition)
                        nc.gpsimd.affine_select(
                            out=expt, in_=expt, compare_op=ALU.is_ge,
                            fill=0.0, base=0, channel_multiplier=-1,
                            pattern=[[-128, 2], [1, 256]])
                        expts.append(expt)
                    # phase 2: attnT + row-sums per head pair
                    recfs, pats = [], []
                    for c in range(KC):
                        recf = apool.tile([1, 2, 256], F32, tag="recf", name="recf")
                        pat = psat.tile([HS + 1, 512], F32, tag="psat", name="psat")
                        for hh in range(2):
                            expt = expts[2 * c + hh]
                            for j in range(2):
                                nc.tensor.matmul(
                                    pat[:, hh * 256:(hh + 1) * 256],
                                    v_sb[:, j, 2 * c + hh, :], expt[:, j, :],
                                    start=(j == 0), stop=(j == 1))
                        nc.vector.reciprocal(
                            out=recf.rearrange("p a b -> p (a b)"),
                            in_=pat[HS:HS + 1, :])
                        recfs.append(recf)
                        pats.append(pat)
                    # phase 3: broadcast 1/s (K=1 matmul) and normalize
                    recrs = []
                    for c in range(KC):
                        recr = apool.tile([1, 2, 256], F32R, tag="recr", name="recr")
                        nc.gpsimd.tensor_copy(out=recr, in_=recfs[c])
                        recrs.append(recr)
                    bcss = []
                    for c in range(KC):
                        pbc = psat.tile([64, 512], F32, tag="psat", name="pbc")
                        nc.tensor.matmul(
                            pbc, ones[:, 0:64],
                            recrs[c].rearrange("o h t -> o (h t)"),
                            start=True, stop=True)
                        bcs = apool.tile([64, 2, 256], F32, tag="bcs", name="bcs")
                        nc.scalar.copy(out=bcs.rearrange("p a b -> p (a b)"),
                                       in_=pbc)
                        bcss.append(bcs)
                    for c in range(KC):
                        for hh in range(2):
                            nc.vector.tensor_mul(
                                out=attn_cat[hh * 64:(hh + 1) * 64, c, :],
                                in0=pats[c][0:HS, hh * 256:(hh + 1) * 256],
                                in1=bcss[c][:, hh, :])
                    # Wo of the PREVIOUS sequence (pipeline)
                    if pending_wo:
                        emit_wo(*pending_wo.pop())
                    pending_wo.append((s, attn_cat))
                if pending_wo:
                    emit_wo(*pending_wo.pop())

                # ---- MLP ----
                h2ts = layer_norm_to_hT(x_sb)
                for g in range(8):
                    h2t = h2ts[g // 2]
                    gw = g % 2
                    h1t = h1pool.tile([128, MC, 256], F32R, tag="h1t", name="h1t")
                    for m2 in range(MC // 2):
                        pm = ps.tile([128, 512], F32, tag="p512", name="p512")
                        for mm in range(2):
                            m = 2 * m2 + mm
                            for k in range(KC):
                                nc.tensor.matmul(
                                    pm[:, mm * 256:(mm + 1) * 256],
                                    w1[:, k, m * 128:(m + 1) * 128],
                                    h2t[:, k, gw * 256:(gw + 1) * 256],
                                    start=(k == 0), stop=(k == KC - 1))
                        if has_b1:
                            for mm in range(2):
                                m = 2 * m2 + mm
                                nc.scalar.activation(
                                    out=h1t[:, m, :], in_=pm[:, mm * 256:(mm + 1) * 256],
                                    func=AF.Relu, bias=b1t[:, l, m:m + 1], scale=1.0)
                        else:
                            nc.scalar.activation(
                                out=h1t[:, 2 * m2:2 * m2 + 2, :], in_=pm,
                                func=AF.Relu)
                    for half in range(2):
                        t = 2 * g + half
                        p2 = ps.tile([128, 512], F32, tag="p512", name="p512")
                        for m in range(MC):
                            nc.tensor.matmul(
                                p2[:, :E], h1t[:, m, half * 128:(half + 1) * 128],
                                w2[:, m, :], start=(m == 0), stop=(m == MC - 1))
                        nc.vector.tensor_add(out=x_sb[:, t, :],
                                             in0=x_sb[:, t, :], in1=p2[:, :E])

            # ---- final LN + unembed ----
            hfts = layer_norm_to_hT(x_sb)
            for t in range(NTILE):
                hft = hfts[t // 4]
                half = t % 4
                pl = ps.tile([128, 512], F32, tag="p512", name="p512")
                for k in range(KC):
                    nc.tensor.matmul(
                        pl[:, :V + 1], hft[:, k, half * 128:(half + 1) * 128],
                        wout[:, k, :], start=(k == 0), stop=(k == KC - 1))
                lsb = spool.tile([128, V], F32, tag="lsb", name="lsb")
                nc.scalar.copy(out=lsb, in_=pl[:, :V])
                nc.sync.dma_start(out=logits_d.ap()[t * 128:(t + 1) * 128, :],
                                  in_=lsb)
            rep_cm.__exit__(None, None, None)

    _legalize_waits(nc)
    return nc


_CACHE = {}


def _get_nc(has_b1):
    if has_b1 not in _CACHE:
        _CACHE[has_b1] = build(has_b1)
    return _CACHE[has_b1]


def kernel(encoding, tok_emb, pos_emb, Wq, Wk, Wv, Wo, bo, W1, b1, W2, b2,
           ln1_s, ln1_b, ln2_s, ln2_b, lnf_s, lnf_b, Wout, bout):
    encoding = np.asarray(encoding)
    f = lambda a: np.ascontiguousarray(np.asarray(a), dtype=np.float32)
    tok_emb, pos_emb = f(tok_emb), f(pos_emb)
    Wq, Wk, Wv, Wo, bo = f(Wq), f(Wk), f(Wv), f(Wo), f(bo)
    W1, b1, W2, b2 = f(W1), f(b1), f(W2), f(b2)
    ln1_s, ln1_b, ln2_s, ln2_b = f(ln1_s), f(ln1_b), f(ln2_s), f(ln2_b)
    lnf_s, lnf_b, Wout, bout = f(lnf_s), f(lnf_b), f(Wout), f(bout)

    # --- host-side folding of LN affine params into adjacent matmuls ---
    # h = x_hat*g + b ; h @ W = x_hat @ (g[:,None]*W) + b @ W.
    def fold_qkv(W, g):
        Wt = W.transpose(0, 2, 1, 3).reshape(L, E, E)     # [L, E, (h d)]
        return Wt * g[:, :, None]

    wq_f = fold_qkv(Wq, ln1_s)
    wk_f = fold_qkv(Wk, ln1_s)
    wv_f = fold_qkv(Wv, ln1_s)
    w1_f = W1 * ln2_s[:, :, None]
    wout_f = Wout * lnf_s[:, None]

    def rank1(beta, W):  # [L,E] x [L,E,F] -> [L,F]
        return np.einsum('le,lef->lf', beta, W)

    q_bias = rank1(ln1_b, wq_f)
    k_bias = rank1(ln1_b, wk_f)
    v_bias = rank1(ln1_b, wv_f)
    w1_bias = rank1(ln2_b, w1_f) + b1
    out_bias = (lnf_b @ wout_f) + bout
    assert not q_bias.any() and not k_bias.any() and not v_bias.any(), \
        "nonzero folded q/k/v bias unsupported by this kernel build"
    assert not bo.any() and not b2.any(), "nonzero bo/b2 unsupported"
    assert not out_bias.any(), "nonzero unembed bias unsupported"
    has_b1 = bool(w1_bias.any())

    wqkv = np.ascontiguousarray(np.stack([wq_f, wk_f, wv_f], axis=1))
    wout_pad = np.zeros((E, V + 1), np.float32)
    wout_pad[:, :V] = wout_f
    nc = _get_nc(has_b1)

    ident_np = np.eye(128, dtype=np.float32)
    onescol_np = np.ones((128, 1), np.float32)
    enc_i = encoding.astype(np.int64)

    in_maps = []
    for c in range(N_CORES):
        enc_c = enc_i[c * SEQ_PER_CORE:(c + 1) * SEQ_PER_CORE].reshape(-1)
        onehot = (np.arange(V)[:, None] == enc_c[None, :]).astype(np.float32)
        m = {
            "onehot": onehot,
            "tokemb": tok_emb,
            "posemb": pos_emb,
            "wqkv": wqkv,
            "wo": Wo,
            "w1": w1_f,
            "w2": W2,
            "wout": wout_pad,
            "ident": ident_np,
            "onescol": onescol_np,
            "ones": np.ones((1, 128), np.float32),
        }
        if has_b1:
            m["b1t"] = np.ascontiguousarray(w1_bias)
        in_maps.append(m)

    res = run_bass_kernel_spmd(nc, in_maps, core_ids=list(range(N_CORES)))
    out = np.concatenate(
        [r["logits"].reshape(SEQ_PER_CORE, T, V) for r in res.results], axis=0)
    return out.astype(np.float32)


# revision 25
# speedup vs baseline: 3989.5014x; 1.0934x over previous
"""Trainium2 Bass kernel for nn_BigramLM_34273839022823.

10-layer dense transformer LM forward: B=64, T=256, E=384, H=6, HS=64, V=65.
Sharding: data-parallel over batch across 8 NeuronCores (8 sequences each),
parameters replicated. No collectives.

Per-core design:
  - 2048 tokens as 16 tiles of 128 (seq s = tiles 2s, 2s+1); residual stream
    x_sb [128, 16, 384] stays SBUF-resident in natural (token-partition) form.
  - Matmuls run on transposed activations (hT [E, tok]) produced by PE
    transposes of the LN output; weights stream from DRAM in natural layout.
  - Attention uses transposed scores sT[tk, tq] = kT.T @ qT so causal mask +
    exp run as one ACT op + one affine_select (scores are tiny, so softmax
    without max-subtraction is exact enough); softmax denominators come from a
    ones-column appended to V; normalization multiplies by a K=1-matmul
    broadcast of 1/s.
  - All matmul inputs are float32r (PE full rate at N>=256).
  - LN scale/bias are folded into adjacent weights host-side; all additive
    bias terms are zero for this model instance (asserted), except b1 which
    is supported via the ACT per-partition bias port on the ReLU.
"""
import sys, os, contextlib

for _p in ("/opt/trn_rl_repo",):
    if _p not in sys.path and os.path.isdir(_p):
        sys.path.insert(0, _p)

import numpy as np

import concourse.bass as bass
import concourse.mybir as mybir
import concourse.tile as tile
from concourse.bass_utils import run_bass_kernel_spmd

F32 = mybir.dt.float32
F32R = mybir.dt.float32r
AF = mybir.ActivationFunctionType
ALU = mybir.AluOpType

N_CORES = 8
B, T, E, H, HS, L, V = 64, 256, 384, 6, 64, 10, 65
E4 = 4 * E                      # 1536
SEQ_PER_CORE = B // N_CORES     # 8
NTOK = SEQ_PER_CORE * T         # 2048
NTILE = NTOK // 128             # 16
KC = E // 128                   # 3 K-chunks over E
MC = E4 // 128                  # 12 M-chunks over 4E
SCALE = float(E) ** -0.5
EPS = 1e-5


def _legalize_waits(nc, max_waits=1):
    """walrus codegen in this toolchain accepts at most one sync wait per
    engine instruction; spill extras onto preceding single-wait NoOps."""
    n_split = 0
    for fn in nc.m.functions:
        for bb in fn.blocks:
            if not any(i.sync_info is not None and len(i.sync_info.on_wait) > max_waits
                       for i in bb.instructions):
                continue
            out = []
            for inst in bb.instructions:
                si = inst.sync_info
                if si is not None and len(si.on_wait) > max_waits:
                    waits = list(si.on_wait)
                    spill, keep = waits[:-max_waits], waits[-max_waits:]
                    for i, w in enumerate(spill):
                        out.append(mybir.InstNoOp(
                            name=f"{inst.name}-wsplit{i}",
                            engine=inst.engine,
                            sync_info=mybir.SyncInfo(on_wait=[w], on_update=[]),
                        ))
                        n_split += 1
                    inst.sync_info = mybir.SyncInfo(
                        on_wait=keep, on_update=list(si.on_update))
                out.append(inst)
            bb.instructions = out
    return n_split


def build(has_b1, reps=1):
    nc = bass.Bass("TRN2", num_devices=N_CORES)

    # ---- DRAM I/O (per core) ----
    onehot_d = nc.dram_tensor("onehot", [V, NTOK], F32R, kind="ExternalInput")
    tokemb_d = nc.dram_tensor("tokemb", [V, E], F32R, kind="ExternalInput")
    posemb_d = nc.dram_tensor("posemb", [T, E], F32R, kind="ExternalInput")
    wqkv_d = nc.dram_tensor("wqkv", [L, 3, E, E], F32R, kind="ExternalInput")
    wo_d = nc.dram_tensor("wo", [L, E, E], F32R, kind="ExternalInput")
    w1_d = nc.dram_tensor("w1", [L, E, E4], F32R, kind="ExternalInput")
    w2_d = nc.dram_tensor("w2", [L, E4, E], F32R, kind="ExternalInput")
    wout_d = nc.dram_tensor("wout", [E, V + 1], F32R, kind="ExternalInput")
    ident_d = nc.dram_tensor("ident", [128, 128], F32R, kind="ExternalInput")
    onescol_d = nc.dram_tensor("onescol", [128, 1], F32R, kind="ExternalInput")
    ones_d = nc.dram_tensor("ones", [1, 128], F32R, kind="ExternalInput")
    b1_d = (nc.dram_tensor("b1t", [L, E4], F32, kind="ExternalInput")
            if has_b1 else None)
    logits_d = nc.dram_tensor("logits", [NTOK, V], F32, kind="ExternalOutput")

    with tile.TileContext(nc) as tc:
        with contextlib.ExitStack() as ctx:
            singles = ctx.enter_context(tc.tile_pool(name="singles", bufs=1))
            wpool = ctx.enter_context(tc.tile_pool(name="wpool", bufs=1))
            hpool = ctx.enter_context(tc.tile_pool(name="hpool", bufs=3))
            h1pool = ctx.enter_context(tc.tile_pool(name="h1pool", bufs=2))
            apool = ctx.enter_context(tc.tile_pool(name="apool", bufs=3))
            spool = ctx.enter_context(tc.tile_pool(name="spool", bufs=2))
            ps = ctx.enter_context(tc.tile_pool(name="ps", bufs=2, space="PSUM"))
            psat = ctx.enter_context(tc.tile_pool(name="psat", bufs=6, space="PSUM"))

            # ---- constants ----
            ident = singles.tile([128, 128], F32R)
            nc.sync.dma_start(out=ident, in_=ident_d.ap())
            onescol = singles.tile([128, 1], F32R)
            nc.sync.dma_start(out=onescol, in_=onescol_d.ap())
            ones = singles.tile([1, 128], F32R)
            nc.sync.dma_start(out=ones, in_=ones_d.ap())
            tokemb = singles.tile([V, E], F32R)
            nc.sync.dma_start(out=tokemb, in_=tokemb_d.ap())
            posemb = singles.tile([128, 2, E], F32R)
            nc.sync.dma_start(out=posemb,
                              in_=posemb_d.ap().rearrange("(h p) e -> p h e", p=128))
            onehot = singles.tile([V, NTOK], F32R)
            nc.sync.dma_start(out=onehot, in_=onehot_d.ap())
            wout = singles.tile([128, KC, V + 1], F32R)
            nc.sync.dma_start(out=wout,
                              in_=wout_d.ap().rearrange("(ko ki) v -> ki ko v", ki=128))
            b1t = None
            if has_b1:
                b1t = singles.tile([128, L, MC], F32)
                nc.sync.dma_start(
                    out=b1t, in_=b1_d.ap().rearrange("l (m p) -> p l m", p=128))

            # persistent residual stream
            x_sb = singles.tile([128, NTILE, E], F32)
            eps_sb = singles.tile([128, 1], F32)
            nc.vector.memset(eps_sb, EPS)

            rep_cm = tc.For_i(0, reps, 1) if reps > 1 else contextlib.nullcontext()
            rep_cm.__enter__()
            # ---- embedding: x = onehot.T @ tokemb + pos_emb ----
            for t in range(NTILE):
                p0 = ps.tile([128, 512], F32, tag="p512", name="p512")
                nc.tensor.matmul(p0[:, :E], onehot[:, t * 128:(t + 1) * 128],
                                 tokemb, start=True, stop=False)
                nc.tensor.matmul(p0[:, :E], ident, posemb[:, t % 2, :],
                                 start=False, stop=True)
                nc.scalar.copy(out=x_sb[:, t, :], in_=p0[:, :E])

            def layer_norm_to_hT(x_src):
                """LN over the free (E) dim of x_src tiles -> list of 8
                transposed f32r tiles [128, KC, 256], one per 256-tok group."""
                mv = spool.tile([128, NTILE, 2], F32, tag="ln_mv", name="ln_mv")
                for t in range(NTILE):
                    st6 = spool.tile([128, 6], F32, tag="ln_st", name="ln_st")
                    nc.vector.bn_stats(out=st6, in_=x_src[:, t, :])
                    nc.vector.bn_aggr(out=mv[:, t, :], in_=st6)
                # rsqrt(var+eps) = exp(-0.5*ln(var+eps)); Ln/Exp share one
                # ACT table set (unlike Sqrt), avoiding 1.3us table reloads.
                lnv = spool.tile([128, NTILE], F32, tag="ln_lnv", name="ln_lnv")
                nc.scalar.activation(out=lnv, in_=mv[:, :, 1], func=AF.Ln,
                                     bias=eps_sb[:, 0:1], scale=1.0)
                rv = spool.tile([128, NTILE], F32, tag="ln_rv", name="ln_rv")
                nc.scalar.activation(out=rv, in_=lnv, func=AF.Exp, scale=-0.5)
                hts = []
                htmps = []
                for t in range(NTILE):
                    htmp = spool.tile([128, E], F32R, tag="htmp", name="htmp",
                                      bufs=5)
                    nc.vector.tensor_scalar(
                        out=htmp, in0=x_src[:, t, :],
                        scalar1=mv[:, t:t + 1, 0], scalar2=rv[:, t:t + 1],
                        op0=ALU.subtract, op1=ALU.mult)
                    htmps.append(htmp)
                for g in range(4):          # 512-token groups (2 seqs)
                    ht = hpool.tile([128, KC, 512], F32R, tag="hT", name="hT")
                    for k in range(KC):
                        ptr = psat.tile([128, 512], F32R, tag="psat", name="ptr")
                        for q in range(4):
                            nc.tensor.transpose(
                                ptr[:, q * 128:(q + 1) * 128],
                                htmps[4 * g + q][:, k * 128:(k + 1) * 128], ident)
                        nc.scalar.copy(out=ht[:, k, :], in_=ptr)
                    hts.append(ht)
                return hts

            for l in range(L):
                # ---- layer weights (natural layout, K-chunked on partitions) --
                def wtile(tag, src_ap, shape, split):
                    w = wpool.tile(shape, F32R, tag=tag, name=tag)
                    nc.sync.dma_start(out=w, in_=src_ap.rearrange(
                        f"({split} ki) f -> ki {split} f", ki=128))
                    return w
                wq = wtile("wq", wqkv_d.ap()[l, 0], [128, KC, E], "ko")
                wk = wtile("wk", wqkv_d.ap()[l, 1], [128, KC, E], "ko")
                wv = wtile("wv", wqkv_d.ap()[l, 2], [128, KC, E], "ko")
                wo = wtile("wo", wo_d.ap()[l], [128, KC, E], "ko")
                w1 = wtile("w1", w1_d.ap()[l], [128, KC, E4], "ko")
                w2 = wtile("w2", w2_d.ap()[l], [128, MC, E], "mo")

                # ---- LN1 -> hT ----
                hts = layer_norm_to_hT(x_sb)

                # ---- attention, per sequence (Wo pipelined one seq behind) --
                qk_pair = {}
                pending_wo = []

                def emit_wo(s, attn_cat):
                    for half in range(2):
                        t = 2 * s + half
                        po = ps.tile([128, 512], F32, tag="p512", name="p512")
                        for k in range(KC):
                            nc.tensor.matmul(
                                po[:, :E],
                                attn_cat[:, k, half * 128:(half + 1) * 128],
                                wo[:, k, :], start=(k == 0), stop=(k == KC - 1))
                        nc.vector.tensor_add(out=x_sb[:, t, :],
                                             in0=x_sb[:, t, :], in1=po[:, :E])

                for s in range(SEQ_PER_CORE):
                    ht = hts[s // 2]
                    sw = s % 2
                    # v (natural, per token tile) with appended ones column
                    v_sb = apool.tile([128, 2, H, HS + 1], F32R,
                                      tag="v_sb", name="v_sb")
                    nc.vector.tensor_copy(
                        out=v_sb[:, :, :, HS:HS + 1],
                        in_=onescol[:, 0:1, None, None]
                        .to_broadcast([128, 2, H, 1]))
                    for j in range(2):
                        pv = ps.tile([128, 512], F32, tag="p512", name="p512")
                        for k in range(KC):
                            nc.tensor.matmul(
                                pv[:, :E],
                                ht[:, k, (sw * 2 + j) * 128:(sw * 2 + j + 1) * 128],
                                wv[:, k, :], start=(k == 0), stop=(k == KC - 1))
                        nc.vector.tensor_copy(
                            out=v_sb[:, j, :, 0:HS],
                            in_=pv[:, :E].rearrange("p (h d) -> p h d", h=H))
                    # qT, kT per head-pair for BOTH seqs of the group, N=512
                    if sw == 0:
                        qk = []
                        for c in range(KC):
                            pq = ps.tile([128, 512], F32, tag="p512", name="p512")
                            for k in range(KC):
                                nc.tensor.matmul(
                                    pq, wq[:, k, c * 128:(c + 1) * 128],
                                    ht[:, k, :], start=(k == 0), stop=(k == KC - 1))
                            pk = ps.tile([128, 512], F32, tag="p512", name="p512")
                            for k in range(KC):
                                nc.tensor.matmul(
                                    pk, wk[:, k, c * 128:(c + 1) * 128],
                                    ht[:, k, :], start=(k == 0), stop=(k == KC - 1))
                            qkt = apool.tile([128, 2, 512], F32R, tag="qkt",
                                             name="qkt")
                            nc.vector.tensor_copy(out=qkt[:, 0, :], in_=pq)
                            nc.vector.tensor_copy(out=qkt[:, 1, :], in_=pk)
                            qk.append(qkt)
                        qk_pair[s // 2] = qk
                    qk = qk_pair[s // 2]

                    attn_cat = apool.tile([128, KC, 256], F32R,
                                          tag="attn_cat", name="attn_cat")
                    # phase 1: all 6 heads' transposed scores
                    expts = []
                    for h in range(H):
                        c, hh = h // 2, h % 2
                        qkt = qk[c]
                        r0 = hh * 64
                        psc = psat.tile([128, 512], F32, tag="psat", name="psat")
                        for j in range(2):
                            nc.tensor.matmul(
                                psc[:, j * 256:(j + 1) * 256],
                                qkt[r0:r0 + 64, 1,
                                    sw * 256 + j * 128:sw * 256 + (j + 1) * 128],
                                qkt[r0:r0 + 64, 0, sw * 256:(sw + 1) * 256],
                                start=True, stop=True)
                        expt = apool.tile([128, 2, 256], F32R,
                                          tag="expt", name="expt", bufs=5)
                        nc.scalar.activation(
                            out=expt.rearrange("p a b -> p (a b)"),
                            in_=psc, func=AF.Exp, scale=SCALE)
                        # keep where tq >= tk  (tk = 128*j + partition)
                        nc.gpsimd.affine_select(
                            out=expt, in_=expt, compare_op=ALU.is_ge,
                            fill=0.0, base=0, channel_multiplier=-1,
                            pattern=[[-128, 2], [1, 256]])
                        expts.append(expt)
                    # phase 2: attnT + row-sums per head pair
                    recfs, pats = [], []
                    for c in range(KC):
                        recf = apool.tile([1, 2, 256], F32R, tag="recf", name="recf")
                        pat = psat.tile([HS + 1, 512], F32, tag="psat", name="psat")
                        for hh in range(2):
                            expt = expts[2 * c + hh]
                            for j in range(2):
                                nc.tensor.matmul(
                                    pat[:, hh * 256:(hh + 1) * 256],
                                    v_sb[:, j, 2 * c + hh, :], expt[:, j, :],
                                    start=(j == 0), stop=(j == 1))
                        with nc.allow_low_precision(reason="1/s rounds to f32r for the broadcast matmul rhs"):
                            nc.vector.reciprocal(
                                out=recf.rearrange("p a b -> p (a b)"),
                                in_=pat[HS:HS + 1, :])
                        recfs.append(recf)
                        pats.append(pat)
                    # phase 3: broadcast 1/s (K=1 matmul) and normalize
                    recrs = recfs
                    bcss = []
                    for c in range(KC):
                        pbc = psat.tile([64, 512], F32, tag="psat", name="pbc")
                        nc.tensor.matmul(
                            pbc, ones[:, 0:64],
                            recrs[c].rearrange("o h t -> o (h t)"),
                            start=True, stop=True)
                        bcs = apool.tile([64, 2, 256], F32, tag="bcs", name="bcs")
                        nc.scalar.copy(out=bcs.rearrange("p a b -> p (a b)"),
                                       in_=pbc)
                        bcss.append(bcs)
                    for c in range(KC):
                        for hh in range(2):
                            nc.vector.tensor_mul(
                                out=attn_cat[hh * 64:(hh + 1) * 64, c, :],
                                in0=pats[c][0:HS, hh * 256:(hh + 1) * 256],
                                in1=bcss[c][:, hh, :])
                    # Wo of the PREVIOUS sequence (pipeline)
                    if pending_wo:
                        emit_wo(*pending_wo.pop())
                    pending_wo.append((s, attn_cat))
                if pending_wo:
                    emit_wo(*pending_wo.pop())

                # ---- MLP ----
                h2ts = layer_norm_to_hT(x_sb)
                for g in range(8):
                    h2t = h2ts[g // 2]
                    gw = g % 2
                    h1t = h1pool.tile([128, MC, 256], F32R, tag="h1t", name="h1t")
                    for m2 in range(MC // 2):
                        pm = ps.tile([128, 512], F32, tag="p512", name="p512")
                        for mm in range(2):
                            m = 2 * m2 + mm
                            for k in range(KC):
                                nc.tensor.matmul(
                                    pm[:, mm * 256:(mm + 1) * 256],
                                    w1[:, k, m * 128:(m + 1) * 128],
                                    h2t[:, k, gw * 256:(gw + 1) * 256],
                                    start=(k == 0), stop=(k == KC - 1))
                        if has_b1:
                            for mm in range(2):
                                m = 2 * m2 + mm
                                nc.scalar.activation(
                                    out=h1t[:, m, :], in_=pm[:, mm * 256:(mm + 1) * 256],
                                    func=AF.Relu, bias=b1t[:, l, m:m + 1], scale=1.0)
                        else:
                            nc.scalar.activation(
                                out=h1t[:, 2 * m2:2 * m2 + 2, :], in_=pm,
                                func=AF.Relu)
                    for half in range(2):
                        t = 2 * g + half
                        p2 = ps.tile([128, 512], F32, tag="p512", name="p512")
                        for m in range(MC):
                            nc.tensor.matmul(
                                p2[:, :E], h1t[:, m, half * 128:(half + 1) * 128],
                                w2[:, m, :], start=(m == 0), stop=(m == MC - 1))
                        nc.vector.tensor_add(out=x_sb[:, t, :],
                                             in0=x_sb[:, t, :], in1=p2[:, :E])

            # ---- final LN + unembed ----
            hfts = layer_norm_to_hT(x_sb)
            for t in range(NTILE):
                hft = hfts[t // 4]
                half = t % 4
                pl = ps.tile([128, 512], F32, tag="p512", name="p512")
                for k in range(KC):
                    nc.tensor.matmul(
                        pl[:, :V + 1], hft[:, k, half * 128:(half + 1) * 128],
                        wout[:, k, :], start=(k == 0), stop=(k == KC - 1))
                lsb = spool.tile([128, V], F32, tag="lsb", name="lsb")
                nc.scalar.copy(out=lsb, in_=pl[:, :V])
                nc.sync.dma_start(out=logits_d.ap()[t * 128:(t + 1) * 128, :],
                                  in_=lsb)
            rep_cm.__exit__(None, None, None)

    _legalize_waits(nc)
    return nc


_CACHE = {}


def _get_nc(has_b1):
    if has_b1 not in _CACHE:
        _CACHE[has_b1] = build(has_b1)
    return _CACHE[has_b1]


def kernel(encoding, tok_emb, pos_emb, Wq, Wk, Wv, Wo, bo, W1, b1, W2, b2,
           ln1_s, ln1_b, ln2_s, ln2_b, lnf_s, lnf_b, Wout, bout):
    encoding = np.asarray(encoding)
    f = lambda a: np.ascontiguousarray(np.asarray(a), dtype=np.float32)
    tok_emb, pos_emb = f(tok_emb), f(pos_emb)
    Wq, Wk, Wv, Wo, bo = f(Wq), f(Wk), f(Wv), f(Wo), f(bo)
    W1, b1, W2, b2 = f(W1), f(b1), f(W2), f(b2)
    ln1_s, ln1_b, ln2_s, ln2_b = f(ln1_s), f(ln1_b), f(ln2_s), f(ln2_b)
    lnf_s, lnf_b, Wout, bout = f(lnf_s), f(lnf_b), f(Wout), f(bout)

    # --- host-side folding of LN affine params into adjacent matmuls ---
    # h = x_hat*g + b ; h @ W = x_hat @ (g[:,None]*W) + b @ W.
    def fold_qkv(W, g):
        Wt = W.transpose(0, 2, 1, 3).reshape(L, E, E)     # [L, E, (h d)]
        return Wt * g[:, :, None]

    wq_f = fold_qkv(Wq, ln1_s)
    wk_f = fold_qkv(Wk, ln1_s)
    wv_f = fold_qkv(Wv, ln1_s)
    w1_f = W1 * ln2_s[:, :, None]
    wout_f = Wout * lnf_s[:, None]

    def rank1(beta, W):  # [L,E] x [L,E,F] -> [L,F]
        return np.einsum('le,lef->lf', beta, W)

    q_bias = rank1(ln1_b, wq_f)
    k_bias = rank1(ln1_b, wk_f)
    v_bias = rank1(ln1_b, wv_f)
    w1_bias = rank1(ln2_b, w1_f) + b1
    out_bias = (lnf_b @ wout_f) + bout
    assert not q_bias.any() and not k_bias.any() and not v_bias.any(), \
        "nonzero folded q/k/v bias unsupported by this kernel build"
    assert not bo.any() and not b2.any(), "nonzero bo/b2 unsupported"
    assert not out_bias.any(), "nonzero unembed bias unsupported"
    has_b1 = bool(w1_bias.any())

    wqkv = np.ascontiguousarray(np.stack([wq_f, wk_f, wv_f], axis=1))
    wout_pad = np.zeros((E, V + 1), np.float32)
    wout_pad[:, :V] = wout_f
    nc = _get_nc(has_b1)

    ident_np = np.eye(128, dtype=np.float32)
    onescol_np = np.ones((128, 1), np.float32)
    enc_i = encoding.astype(np.int64)

    in_maps = []
    for c in range(N_CORES):
        enc_c = enc_i[c * SEQ_PER_CORE:(c + 1) * SEQ_PER_CORE].reshape(-1)
        onehot = (np.arange(V)[:, None] == enc_c[None, :]).astype(np.float32)
        m = {
            "onehot": onehot,
            "tokemb": tok_emb,
            "posemb": pos_emb,
            "wqkv": wqkv,
            "wo": Wo,
            "w1": w1_f,
            "w2": W2,
            "wout": wout_pad,
            "ident": ident_np,
            "onescol": onescol_np,
            "ones": np.ones((1, 128), np.float32),
        }
        if has_b1:
            m["b1t"] = np.ascontiguousarray(w1_bias)
        in_maps.append(m)

    res = run_bass_kernel_spmd(nc, in_maps, core_ids=list(range(N_CORES)))
    out = np.concatenate(
        [r["logits"].reshape(SEQ_PER_CORE, T, V) for r in res.results], axis=0)
    return out.astype(np.float32)
